# revision 4
# baseline (speedup 1.0000x reference)
"""Trainium2 Bass kernel for AttentionPooling.

v2 changes over the first working version (86-91us -> ~76-79us):
  - x shipped host-pretransposed [BPC, 128, NCHUNK, N] and loaded with ONE
    4MB dma_start per batch (32KB contiguous per partition) -> ~88-93% of
    the 358 GB/s per-core HBM rate (was 8x512KB at ~75%).
  - Steady-state loop body refills each batch's x tile in place right
    after its readers finish (For_i ends every iteration with an
    all-engine barrier, so cross-iteration prefetch is impossible); the
    last batch refills in quarters on the ACT HWDGE ring so the final
    piece doesn't serialize ~13us of DMA behind the barrier.
  - Cold start: batch 0/1 prologue DMAs split into quarters/halves so the
    PE starts ~3.5us in and never starves.
  - Attention lhsT padded with a ones block to exactly 128 columns, which
    enables fast-weight-load (measured attention pair ~113 -> ~77ns).
  - Last tile of the last batch copies attention operands per-subtile so
    the finalize chain starts earlier.
  - Biases folded into the attention operands during the DVE staging op
    (tensor_tensor ADD with a materialized [128,NSUB,192] bias tile): the
    attention matmul then yields fully-biased logits and v-sums, deleting
    the analytic bias-correction chain (skp/sq_sb/sq broadcast/L1/LT) and
    letting softmax read the PSUM logits directly.  NOTE: DVE
    tensor_tensor does NOT honor stride-0 broadcast APs on its second
    operand — the bias tile must be materialized at full shape.

Measured decomposition per loop iteration (p25, one core, 4 batches):
  proj matmuls alone            ~51us   (512 pairs x ~100ns; stream 80ns
                                         + ~20ns unhidable LDW/issue)
  + attention matmuls + copies  ~61us   (128 pairs x ~77ns)
  + finalize chains + tail      ~72us
  DMA alone                     ~52us   (fully overlapped except the tail)
  full                          ~76-79us

Math (per batch element b):
  xf = x[b] reshaped [C, N] with C=512, N=4096
  q = wq@xf + bq ; k = wk@xf + bk ; v = wv@xf + bv          (each [64, N])
  logits = q @ k^T  [64, 64];  attn = softmax(logits, axis over rows o)
  out[b] = mean_n(attn @ v) = attn @ mean_n(v)              ([64])

Because attn does not depend on n, mean_n(attn @ v) = attn @ vbar with
vbar = mean_n(v) — the heavy [64, N] attn@v product collapses to a [64]
vector, so only the q/k projections and a 64x64 logits product are real
work.  Implementation, per batch element:

  - x, wq, wk, wv are shipped as fp16 (10-bit mantissa, same class as
    tf32/f32r device rounding; empirically 4.5e-3 end-to-end rel err)
    which halves the HBM traffic for x — the dominant memory cost.
  - X-STATIONARY fused projection: each [128, 128] x chunk-subtile is
    the matmul weight; ONE matmul per (chunk, subtile) streams the fused
    [wqT | wvT | wkT] 192-column block through it, producing
    [q0T | v0T | k0T] directly in n-major layout (no transposes, half
    the PE instructions), fp32 PSUM accumulation over the 4 C-chunks.
  - One 3D-AP vector-engine copy per projection tile moves all 4
    subtiles' [ones | qT | vT | kT | ones] attention operands to SBUF;
    the ones-augmented [65, 129] attention matmuls (lhsT=[kT|1],
    rhs=[1|qT|vT], accumulated over all 32 n-subtiles) produce L0^T AND
    sum_n k0 (column 0), sum_n q0, sum_n v0 (partition-64 row) — the
    bias-correction and pooling sums ride the same accumulation.  They
    run one projection tile behind so they never stall on the copies.
  - The v-sum row is transposed off PSUM partition 64 with a rank-1
    matmul whose operands both live on partition 64.
  - Softmax along the free dim of L^T (scalar-engine exp with accumulated
    denominator), folded:  out = E^T @ (vbar / s) as one [64]x[64,64]
    matmul producing the output row directly.

Data-parallel over batch across the 8 NeuronCores (4 batch elements per
core); no collectives needed.
"""

import sys

import numpy as np

for _p in ("/opt/trn_rl_repo", "/root/.axon_site/_ro/trn_rl_repo"):
    if _p not in sys.path:
        sys.path.insert(0, _p)

import concourse.bacc as bacc
import concourse.mybir as mybir
import concourse.tile as tile
from concourse import masks
from concourse.bass_utils import run_bass_kernel_spmd

B, C, H, W = 32, 512, 64, 64
N = H * W            # 4096
C8 = 64              # C // 8
NCORES = 8
BPC = B // NCORES    # batch elements per core
NCHUNK = C // 128    # C chunks of 128
TW = 512             # projection tile width (PSUM bank = 512 f32)
NT = N // TW         # 8 projection tiles
NSUB = TW // 128     # transpose subtiles per projection tile

F32 = mybir.dt.float32
F32R = mybir.dt.float32r
F16 = mybir.dt.float16
AX = mybir.AxisListType.X
MULT = mybir.AluOpType.mult
ADD = mybir.AluOpType.add

_NC_CACHE = {}


def _build_nc(loop_n=None, mode="full"):
    """Build the bass program.  loop_n wraps the per-batch section in a
    device-side For_i loop (used only for timing: the NEFF then executes the
    whole workload loop_n times back-to-back, making device time measurable
    over the host dispatch overhead).  mode: "full" | "dma" (x loads only)
    | "compute" (batch-0 x loaded once outside the loop, engines only)."""
    nc = bacc.Bacc("TRN2", target_bir_lowering=False, debug=False)

    x_d = nc.dram_tensor("x", [BPC, 128, NCHUNK, N], F16, kind="ExternalInput")
    wq_d = nc.dram_tensor("wq", [C8, C], F16, kind="ExternalInput")
    bq_d = nc.dram_tensor("bq", [C8], F32, kind="ExternalInput")
    wk_d = nc.dram_tensor("wk", [C8, C], F16, kind="ExternalInput")
    bk_d = nc.dram_tensor("bk", [C8], F32, kind="ExternalInput")
    wv_d = nc.dram_tensor("wv", [C8, C], F16, kind="ExternalInput")
    bv_d = nc.dram_tensor("bv", [C8], F32, kind="ExternalInput")
    out_d = nc.dram_tensor("out", [BPC, C8], F32, kind="ExternalOutput")

    with tile.TileContext(nc, trace_sim=False) as tc:
        with (
            tc.tile_pool(name="const", bufs=1) as constp,
            tc.tile_pool(name="xpool", bufs=1) as xpool,
            tc.tile_pool(name="qkpool", bufs=4) as qkpool,
            tc.tile_pool(name="attpool", bufs=4) as attpool,
            tc.tile_pool(name="smallp", bufs=2) as smallp,
            tc.tile_pool(name="ps_qk", bufs=2, space="PSUM") as ps_qk,
            tc.tile_pool(name="ps_att", bufs=3, space="PSUM") as ps_att,
            tc.tile_pool(name="ps_small", bufs=1, space="PSUM") as ps_small,
            tc.tile_pool(name="ps_warm", bufs=1, space="PSUM") as ps_warm,
        ):
            # ---------------- one-time prep ----------------
            ident = constp.tile([128, 128], F32)
            masks.make_identity(nc, ident[:])
            ident16 = constp.tile([128, 128], F16)
            nc.scalar.copy(ident16[:], ident[:])

            ones_row = constp.tile([1, C8], F32)
            nc.vector.memset(ones_row[:], 1.0)
            ones2_f32 = constp.tile([128, 2], F32)
            nc.vector.memset(ones2_f32[:], 1.0)
            ones2_16 = constp.tile([128, 2], F16)
            nc.scalar.copy(ones2_16[:], ones2_f32[:])
            ones_pad_f32 = constp.tile([128, 65], F32)
            nc.vector.memset(ones_pad_f32[:], 1.0)
            ones_pad16 = constp.tile([128, 65], F16)
            nc.scalar.copy(ones_pad16[:], ones_pad_f32[:])
            # ones at partition 64 (to broadcast the sq row the attention
            # matmul leaves on PSUM partition 64)
            ones64 = constp.tile([C8 + 1, C8], F32)
            nc.vector.memset(ones64[C8 : C8 + 1, :], 1.0)

            wq_raw = constp.tile([C8, C], F16)
            nc.sync.dma_start(wq_raw[:], wq_d.ap()[:, :])
            wk_raw = constp.tile([C8, C], F16)
            nc.sync.dma_start(wk_raw[:], wk_d.ap()[:, :])
            wv_raw = constp.tile([C8, C], F16)
            nc.sync.dma_start(wv_raw[:], wv_d.ap()[:, :])

            bq_row = constp.tile([1, C8], F32)
            nc.sync.dma_start(bq_row[:], bq_d.ap().unsqueeze(0))
            bk_row = constp.tile([1, C8], F32)
            nc.sync.dma_start(bk_row[:], bk_d.ap().unsqueeze(0))
            bv_row = constp.tile([1, C8], F32)
            nc.sync.dma_start(bv_row[:], bv_d.ap().unsqueeze(0))

            # fused transposed weight chunks: wqkvT[c] = [wqT | wvT | wkT]
            wqkvT = []
            for c in range(NCHUNK):
                csl = slice(c * 128, (c + 1) * 128)
                pt = ps_small.tile([128, 192], F16, tag="sp")
                nc.tensor.transpose(
                    pt[:, 0:C8], wq_raw[:, csl], ident16[0:C8, 0:C8]
                )
                nc.tensor.transpose(
                    pt[:, C8 : 2 * C8], wv_raw[:, csl], ident16[0:C8, 0:C8]
                )
                nc.tensor.transpose(
                    pt[:, 2 * C8 : 192], wk_raw[:, csl], ident16[0:C8, 0:C8]
                )
                st = constp.tile([128, 192], F16, tag=f"wqkvT{c}")
                nc.scalar.copy(st[:], pt[:])
                wqkvT.append(st)

            # bias matrix for folding biases into the attention operands:
            # every partition row = [bq | bv | bk] (matches wqkvT order)
            ones_row128 = constp.tile([1, 128], F32)
            nc.vector.memset(ones_row128[:], 1.0)
            bias_row = constp.tile([1, 192], F32)
            nc.scalar.copy(bias_row[:, 0:C8], bq_row[:])
            nc.scalar.copy(bias_row[:, C8 : 2 * C8], bv_row[:])
            nc.scalar.copy(bias_row[:, 2 * C8 : 192], bk_row[:])
            b_ps = ps_small.tile([128, 192], F32, tag="sp")
            nc.tensor.matmul(b_ps[:], ones_row128[:], bias_row[:], start=True, stop=True)
            bias_nsub = constp.tile([128, NSUB, 192], F32)
            for _s in range(NSUB):
                nc.scalar.copy(bias_nsub[:, _s, :], b_ps[:])
            zero_col = constp.tile([C8, 1], F32)
            nc.vector.memset(zero_col[:], 0.0)

            # bias-derived constants
            p_bc = ps_small.tile([C8, C8], F32, tag="sp")
            nc.tensor.matmul(p_bc[:], ones_row[:], bq_row[:], start=True, stop=True)
            bq_bc = constp.tile([C8, C8], F32)  # every row = bq
            nc.scalar.copy(bq_bc[:], p_bc[:])

            p_bk = ps_small.tile([C8, 1], F32, tag="sp")
            nc.tensor.matmul(
                p_bk[:], bk_row[:], ones_row[:, 0:1], start=True, stop=True
            )
            bk_col = constp.tile([C8, 1], F32)
            nc.scalar.copy(bk_col[:], p_bk[:])

            p_bv = ps_small.tile([C8, 1], F32, tag="sp")
            nc.tensor.matmul(
                p_bv[:], bv_row[:], ones_row[:, 0:1], start=True, stop=True
            )
            bv_col = constp.tile([C8, 1], F32)
            nc.scalar.copy(bv_col[:], p_bv[:])

            # ---------------- per batch element ----------------
            def dma_batch_into(b, pool, tagp, nh):
                # one [128, NCHUNK, N] tile per batch, loaded with a single
                # 4MB dma_start (partition p <- channel chunk*128+p); large
                # transfers run near HBM line rate
                t = pool.tile([128, NCHUNK, N], F16, tag=f"{tagp}{b % 4}")
                nc.sync.dma_start(t[:], x_d.ap()[b])
                xc = [[t[:, c, :]] for c in range(NCHUNK)]
                return t, xc

            def dma_refill(b, t):
                # overwrite batch b's x tile in place (the loop reloads the
                # same data; only the steady-state timing matters)
                nc.sync.dma_start(t[:], x_d.ap()[b])

            xc_static = None
            a_static = None
            if mode == "attn":
                a_static = constp.tile([128, NSUB, 256], F16, tag="a_static")
                nc.vector.memset(a_static[:], 0.01)
            if mode in ("proj", "projrt", "projsame", "projg", "projattn"):
                xc_static = dma_batch_into(0, constp, "xs", 1)[1]
            if mode in ("compute", "computert"):
                xc_static = dma_batch_into(0, constp, "xs", 1)[1]

            def dma_batch(b):
                return dma_batch_into(b, xpool, "x", 1)

            def emit_proj_only(b, xc):
                for ti in range(NT):
                    base = ti * TW
                    qk_ps = ps_qk.tile([128, NSUB, 256], F32, tag="qk_ps")
                    for s in range(NSUB):
                        nsl = slice(base + s * 128, base + (s + 1) * 128)
                        for c in range(NCHUNK):
                            nc.tensor.matmul(
                                qk_ps[:, s, 0:192],
                                xc[c][0][:, nsl],
                                wqkvT[c][:],
                                start=(c == 0),
                                stop=(c == NCHUNK - 1),
                            )

            def emit_proj_rt(b, xc):
                for ti in range(NT):
                    base = ti * TW
                    qk_ps = ps_qk.tile([128, NSUB, 256], F32, tag="qk_ps")
                    for s in range(NSUB):
                        nsl = slice(base + s * 128, base + (s + 1) * 128)
                        for c in range(NCHUNK):
                            for h in range(2):
                                hp = slice(h * 64, (h + 1) * 64)
                                nc.tensor.matmul(
                                    qk_ps[:, s, 0:192],
                                    xc[c][0][hp, nsl],
                                    wqkvT[c][hp, :],
                                    start=(c == 0 and h == 0),
                                    stop=(c == NCHUNK - 1 and h == 1),
                                    tile_position=(h * 64, 0),
                                )

            def emit_proj_same(b, xc):
                # probe: identical stationary for every matmul (tests LDW
                # dedup / whether per-MM overhead is the weight load)
                for ti in range(NT):
                    qk_ps = ps_qk.tile([128, NSUB, 256], F32, tag="qk_ps")
                    for s in range(NSUB):
                        for c in range(NCHUNK):
                            nc.tensor.matmul(
                                qk_ps[:, s, 0:192],
                                xc[0][0][:, 0:128],
                                wqkvT[c][:],
                                start=(c == 0),
                                stop=(c == NCHUNK - 1),
                            )

            def emit_proj_g(b, xc):
                # probe: one 16-MM accumulation group per tile (tests PSUM
                # group-boundary cost; numerics are wrong on purpose)
                for ti in range(NT):
                    base = ti * TW
                    qk_ps = ps_qk.tile([128, NSUB, 256], F32, tag="qk_ps")
                    n = 0
                    for s in range(NSUB):
                        nsl = slice(base + s * 128, base + (s + 1) * 128)
                        for c in range(NCHUNK):
                            nc.tensor.matmul(
                                qk_ps[:, 0, 0:192],
                                xc[c][0][:, nsl],
                                wqkvT[c][:],
                                start=(n == 0),
                                stop=(n == 15),
                            )
                            n += 1

            def emit_projattn(b, xc):
                # proj + copies + attention accumulation, no finalize: used
                # to attribute compute time between the attention pipeline
                # and the finalize/tail chain
                pending = None
                att_ps = ps_att.tile([128, 2 * C8 + 1], F32)
                for ti in range(NT):
                    base = ti * TW
                    qk_ps = ps_qk.tile([128, NSUB, 256], F32, tag="qk_ps")
                    for s in range(NSUB):
                        nsl = slice(base + s * 128, base + (s + 1) * 128)
                        for c in range(NCHUNK):
                            nc.tensor.matmul(
                                qk_ps[:, s, 0:192],
                                xc[c][0][:, nsl],
                                wqkvT[c][:],
                                start=(c == 0),
                                stop=(c == NCHUNK - 1),
                            )
                    a_sb = attpool.tile([128, NSUB, 258], F16, tag="a_sb")
                    nc.vector.tensor_copy(a_sb[:, :, 1:193], qk_ps[:, :, 0:192])
                    nc.vector.tensor_copy(
                        a_sb[:, :, 0:1],
                        ones2_16[:, 0:1].unsqueeze(1).broadcast_to([128, NSUB, 1]),
                    )
                    nc.vector.tensor_copy(
                        a_sb[:, :, 193:258],
                        ones_pad16[:].unsqueeze(1).broadcast_to([128, NSUB, 65]),
                    )
                    if pending is not None:
                        emit_attn(*pending)
                    pending = (att_ps, ti, a_sb)
                if pending is not None:
                    emit_attn(*pending)

            def emit_attn_only(b, a_static):
                att_ps = ps_att.tile([C8 + 1, 2 * C8 + 1], F32)
                for ti in range(NT):
                    emit_attn(att_ps, ti, a_static)

            _xts = {}
            _xcs_cache = {}

            def emit_batches_full(proj_rt, refill):
                # steady-state loop body: batch b's compute reads x tile b,
                # then immediately re-issues the DMA that refills the same
                # tile in place for the NEXT loop iteration -> the DMA engine
                # always has ~3 batches of lookahead and the PE never waits
                # on x.  (Prologue DMAs outside the loop fill the tiles.)
                # The LAST batch refills in quarters, each issued as soon as
                # its two tiles are consumed: every For_i iteration ends in
                # an all-engine barrier, so the final refill piece otherwise
                # serializes ~13us of DMA after the compute.
                def mk_hook(b, t):
                    if not refill or b < BPC - 1:
                        return None
                    qw = N // 4

                    def hook(ti):
                        if ti % 2 == 1:
                            q = ti // 2
                            nc.scalar.dma_start(
                                t[:, :, q * qw : (q + 1) * qw],
                                x_d.ap()[b][
                                    :, :, q * qw : (q + 1) * qw
                                ],
                            )

                    return hook

                fin = None
                for b in range(BPC):
                    fin = emit_batch(
                        b,
                        _xcs_cache[b],
                        fin,
                        proj_rt,
                        after_tile=mk_hook(b, _xts[b]),
                    )
                    if refill and b < BPC - 1:
                        dma_refill(b, _xts[b])
                fin()

            def emit_batches():
                if mode == "proj":
                    for b in range(BPC):
                        emit_proj_only(b, xc_static)
                    return
                if mode == "projrt":
                    for b in range(BPC):
                        emit_proj_rt(b, xc_static)
                    return
                if mode == "projsame":
                    for b in range(BPC):
                        emit_proj_same(b, xc_static)
                    return
                if mode == "projg":
                    for b in range(BPC):
                        emit_proj_g(b, xc_static)
                    return
                if mode == "projattn":
                    for b in range(BPC):
                        emit_projattn(b, xc_static)
                    return
                if mode == "attn":
                    for b in range(BPC):
                        emit_attn_only(b, a_static)
                    return
                if mode == "dma":
                    for b in range(BPC):
                        dma_refill(b, _xts[b])
                    return
                if mode == "compute":
                    fin = None
                    for b in range(BPC):
                        fin = emit_batch(b, xc_static, fin, False)
                    fin()
                    return
                if mode == "computert":
                    fin = None
                    for b in range(BPC):
                        fin = emit_batch(b, xc_static, fin, True)
                    fin()
                    return
                emit_batches_full(
                    proj_rt=(mode == "fullrt"), refill=(loop_n is not None)
                )

            def emit_attn(att_ps, ti, a_list):
                for s in range(NSUB):
                    first = ti == 0 and s == 0
                    last = ti == NT - 1 and s == NSUB - 1
                    # lhsT=[kT+bk | ones-pad to 128 cols], rhs=[qT+bq|vT+bv]:
                    #   [0:64, 0:64] = L^T (fully biased), [64, 64:128] =
                    #   sum_n (v0+bv); rows 65:128 duplicate row 64 (pad).
                    # The 128-col stationary enables fast-weight-load.
                    nc.tensor.matmul(
                        att_ps[:],
                        a_list[:, s, 128:256],
                        a_list[:, s, 0:128],
                        start=first,
                        stop=last,
                    )

            def emit_warm(n):
                wp = ps_warm.tile([128, 128], F32, tag="warm")
                for _ in range(n):
                    nc.tensor.matmul(
                        wp[:], ident16[:], ident16[:], start=True, stop=True
                    )

            def emit_batch(b, xc, fin_prev, proj_rt=False, after_tile=None):
                pending = None

                # [128, 128]: [0:64,0:64]=L^T biased, [64,64:128]=sum_n v
                # (accumulated over all subtiles); rows 65:128 pad (unused)
                att_ps = ps_att.tile([128, 2 * C8], F32)

                for ti in range(NT):
                    hh = 0
                    base = ti * TW
                    # x-stationary fused projection: one matmul per
                    # (chunk, subtile) streams [wqT | wvT | wkT] through the
                    # stationary x chunk -> [qT | vT | kT] in n-major layout
                    qk_ps = ps_qk.tile([128, NSUB, 256], F32, tag="qk_ps")
                    for s in range(NSUB):
                        nsl = slice(base + s * 128, base + (s + 1) * 128)
                        for c in range(NCHUNK):
                            if proj_rt:
                                for h in range(2):
                                    hp = slice(h * 64, (h + 1) * 64)
                                    nc.tensor.matmul(
                                        qk_ps[:, s, 0:192],
                                        xc[c][hh][hp, nsl],
                                        wqkvT[c][hp, :],
                                        start=(c == 0 and h == 0),
                                        stop=(c == NCHUNK - 1 and h == 1),
                                        tile_position=(h * 64, 0),
                                    )
                            else:
                                nc.tensor.matmul(
                                    qk_ps[:, s, 0:192],
                                    xc[c][hh][:, nsl],
                                    wqkvT[c][:],
                                    start=(c == 0),
                                    stop=(c == NCHUNK - 1),
                                )

                    # one [128, 4x256] tile holds all 4 subtiles' attention
                    # operands [qT+bq | vT+bv | kT+bk | ones-pad(64)]: the
                    # biases are folded here, so the attention matmul
                    # produces fully-biased logits and the v-sum directly
                    a_sb = attpool.tile([128, NSUB, 256], F16, tag="a_sb")
                    if b == BPC - 1 and ti == NT - 1:
                        for s in range(NSUB):
                            nc.vector.tensor_tensor(
                                a_sb[:, s, 0:192],
                                qk_ps[:, s, 0:192],
                                bias_nsub[:, s, :],
                                op=ADD,
                            )
                    else:
                        nc.vector.tensor_tensor(
                            a_sb[:, :, 0:192],
                            qk_ps[:, :, 0:192],
                            bias_nsub[:],
                            op=ADD,
                        )
                    nc.vector.tensor_copy(
                        a_sb[:, :, 192:256],
                        ones_pad16[:, 0:64]
                        .unsqueeze(1)
                        .broadcast_to([128, NSUB, 64]),
                    )
                    a_list = a_sb
                    # attention matmuls run one projection tile behind, so
                    # their a_sb inputs were copied a whole tile ago (no PE
                    # stall on the DVE copy)
                    if pending is not None:
                        emit_attn(*pending)
                    pending = (att_ps, ti, a_list)
                    if ti == 1 and fin_prev is not None:
                        # previous batch's finalize chain runs here: its
                        # inputs completed a full tile ago, so the PE ops
                        # inside it never stall the engine
                        fin_prev()
                    if after_tile is not None:
                        after_tile(ti)

                if pending is not None:
                    emit_attn(*pending)
                    pending = None

                # only precursor left: stage the v-sum row off PSUM
                vrow_sb = smallp.tile([C8 + 1, C8], F32, tag="vrow_sb")
                nc.scalar.copy(
                    vrow_sb[C8 : C8 + 1, :], att_ps[C8 : C8 + 1, C8 : 2 * C8]
                )

                return lambda: finalize_batch(b, att_ps, vrow_sb)

            def finalize_batch(b, att_ps, vrow_sb):
                # vsum row (partition 64) -> column via rank-1 matmul at p64
                vb_ps = ps_small.tile([C8, 1], F32, tag="sp")
                nc.tensor.matmul(
                    vb_ps[:],
                    vrow_sb[C8 : C8 + 1, :],
                    ones64[C8 : C8 + 1, 0:1],
                    start=True,
                    stop=True,
                )
                vbar = smallp.tile([C8, 1], F32, tag="vbar")
                nc.vector.scalar_tensor_tensor(
                    vbar[:], vb_ps[:], 1.0 / N, zero_col[:], op0=MULT, op1=ADD
                )
                # softmax along the free dim, straight from the PSUM logits
                negm = smallp.tile([C8, 1], F32, tag="negm")
                nc.vector.reduce_max(
                    negm[:], att_ps[0:C8, 0:C8], axis=AX, negate=True
                )
                E = smallp.tile([C8, C8], F32, tag="E")
                s_col = smallp.tile([C8, 1], F32, tag="s_col")
                nc.scalar.activation(
                    E[:],
                    att_ps[0:C8, 0:C8],
                    mybir.ActivationFunctionType.Exp,
                    bias=negm[:],
                    scale=1.0,
                    accum_out=s_col[:],
                )
                # w = vbar / s ; out = E^T @ w  (as row via lhsT=w)
                rs = smallp.tile([C8, 1], F32, tag="rs")
                nc.vector.reciprocal(rs[:], s_col[:])
                wcol = smallp.tile([C8, 1], F32, tag="wcol")
                nc.vector.tensor_tensor(wcol[:], vbar[:], rs[:], op=MULT)
                out_ps = ps_small.tile([1, C8], F32, tag="sp")
                nc.tensor.matmul(out_ps[:], wcol[:], E[:], start=True, stop=True)
                out_row = smallp.tile([1, C8], F32, tag="out_row")
                nc.scalar.copy(out_row[:], out_ps[:])
                # ACT HWDGE ring: idle (x refills are on the SP ring) and
                # ~0.6us first-byte vs ~1us for the gpsimd SWDGE path
                nc.scalar.dma_start(out_d.ap()[b : b + 1, :], out_row[:])

            if mode in ("full", "fullrt", "dma"):
                # cold start: batches 0/1 arrive in quarters/halves so the
                # PE can begin ~3.5us in and never starves while the DMA
                # stream catches up (delivery 13.1us/batch < use 18.5)
                npieces = {0: 4, 1: 2}
                for b in range(BPC):
                    pieces = npieces.get(b, 1)
                    t = xpool.tile([128, NCHUNK, N], F16, tag=f"x{b}")
                    pw = N // pieces
                    for q in range(pieces):
                        nc.sync.dma_start(
                            t[:, :, q * pw : (q + 1) * pw],
                            x_d.ap()[b][:, :, q * pw : (q + 1) * pw],
                        )
                    _xts[b] = t
                    _xcs_cache[b] = [[t[:, c, :]] for c in range(NCHUNK)]

            if loop_n is None:
                emit_batches()
            else:
                hints = (
                    mybir.EngineType.PE,
                    mybir.EngineType.DVE,
                    mybir.EngineType.Activation,
                    mybir.EngineType.SP,
                    mybir.EngineType.Pool,
                )
                with tc.For_i(0, loop_n, 1, hint_engines=hints):
                    emit_batches()

    nc.compile()
    return nc


def _get_nc(loop_n=None, mode="full"):
    key = ("nc", loop_n, mode)
    if key not in _NC_CACHE:
        _NC_CACHE[key] = _build_nc(loop_n, mode)
    return _NC_CACHE[key]


def _make_in_maps(x, wq, bq, wk, bk, wv, bv):
    # fp16 shipping: same 10-bit mantissa as the tf32-class device compute,
    # but halves the HBM traffic for x
    xf = np.ascontiguousarray(
        np.asarray(x, dtype=np.float32)
        .reshape(B, NCHUNK, 128, N)
        .transpose(0, 2, 1, 3)
        .astype(np.float16)
    )
    shared = {
        "wq": np.asarray(wq, np.float32).astype(np.float16),
        "bq": np.asarray(bq, np.float32),
        "wk": np.asarray(wk, np.float32).astype(np.float16),
        "bk": np.asarray(bk, np.float32),
        "wv": np.asarray(wv, np.float32).astype(np.float16),
        "bv": np.asarray(bv, np.float32),
    }
    return [
        {"x": xf[i * BPC : (i + 1) * BPC], **shared} for i in range(NCORES)
    ]


def kernel(x, wq, bq, wk, bk, wv, bv):
    nc = _get_nc()
    in_maps = _make_in_maps(x, wq, bq, wk, bk, wv, bv)
    res = run_bass_kernel_spmd(nc, in_maps, core_ids=list(range(NCORES)))
    out = np.concatenate([res.results[i]["out"] for i in range(NCORES)], axis=0)
    return out.astype(np.float32)

